# revision 25
# baseline (speedup 1.0000x reference)
"""Trainium2 Bass kernel for nn_AttentionBlock (64, 512, 16) / three 8192x8192 Linears.

Strategy (8 NeuronCores, single NEFF, one launch):
  Phase 1 (QKV projection, column-sharded, fp8):
    Each core c owns output columns [1024c, 1024(c+1)) of each Linear
    (= w positions [64c, 64(c+1)), all 16 d). Weights are pre-transposed,
    pre-scaled by 64 (keeps N(0, 0.02^2) values out of e4m3 subnormals)
    and pre-swizzled on host so every 256KB DMA tile is contiguous.
    x is also cast to e4m3. Projection matmuls run in fp8 DoubleRow mode
    (two 128-row k-tiles per instruction). Bias rides a bf16 K=1
    ones-matmul into the same PSUM accumulation. q/k are permuted to
    [b][d][w_local] layout on the way out of PSUM; v stays [b][w_local][d].
  AllToAll: core c sends batch-block j of its q/k (then v) shard to
    core j. After A2A each core holds the full q/k/v for its own 8
    batches -- the attention program is fully static per core.
  Phase 2 (attention, batch-sharded, 8 batches/core):
    alphas are built transposed [k, q] so softmax over the *query* axis
    is a free-dim reduction (exp via ScalarE with fused accum row-sum;
    the 64^2 weight pre-scale folds into the exp input scale). The
    softmax reciprocal AND the leftover 1/64 v-scale fold into the exp'd
    alphas via one two-op tensor_scalar during the overlapped window.
    After the v AllToAll the second einsum consumes v tiles directly as
    matmul lhsT; per-batch results pack into one [128, 512] tile so the
    tail is a single sigmoid + residual add + one 256KB output DMA.
  Host: gathers per-core (128, 512) outputs, transposes back.
"""

import math

import numpy as np
import ml_dtypes

import concourse.bass as bass
import concourse.bacc as bacc
import concourse.mybir as mybir
import concourse.tile as tile
import concourse.bass_utils as bass_utils

N_CORES = 8
BS, W_DIM, D = 64, 512, 16
K = W_DIM * D            # 8192 contraction dim
CPC = K // N_CORES       # 1024 output cols per core
WPC = W_DIM // N_CORES   # 64 w positions per core
BPC = BS // N_CORES      # 8 batches per core
NKT = K // 128           # 64 k-tiles
CHUNK = 8                # k-tiles per weight DMA (4 DoubleRow pairs)
NCH = NKT // CHUNK       # 8 weight chunks
WSCALE = 64.0            # host-side weight pre-scale (fp8 subnormal dodge)
EXP_SCALE = 1.0 / (math.sqrt(K) * WSCALE * WSCALE)

_CACHE: dict = {}


def _build(wbufs: int = 5):
    f8 = mybir.dt.float8e4
    bf16 = mybir.dt.bfloat16
    f32 = mybir.dt.float32
    DR = mybir.MatmulPerfMode.DoubleRow

    nc = bacc.Bacc("TRN2", target_bir_lowering=False, debug=False,
                   num_devices=N_CORES)

    # xt is host-preswizzled to [128 p, 64 kt, 64 b] so the load is one
    # fully contiguous DMA.
    xt_d = nc.dram_tensor("xt", [128, NKT * BS], f8, kind="ExternalInput")
    # weights host-swizzled to [m, p, j, c] -> each 128-row slice is one
    # contiguous 256KB chunk whose rows are 2KB.
    w_d = [nc.dram_tensor(n, [NCH * 128, CHUNK * CPC], f8,
                          kind="ExternalInput")
           for n in ("wq", "wk", "wv")]
    b_d = [nc.dram_tensor(n, [1, CPC], bf16, kind="ExternalInput")
           for n in ("bq", "bk", "bv")]
    # residual x^T, padded to 32-partition quadrants (4 batches per tile,
    # batch q at partitions [32q, 32q+16))
    xtp_d = nc.dram_tensor("xtp", [2 * 128, W_DIM], f32, kind="ExternalInput")
    out_d = nc.dram_tensor("out", [BPC * D, W_DIM], f32, kind="ExternalOutput")

    hwdge = [nc.sync, nc.scalar]          # the two independent HWDGE rings

    with tile.TileContext(nc) as tc:
        with (
            tc.tile_pool(name="constp", bufs=1) as constp,
            tc.tile_pool(name="sbp", bufs=1) as sbp,
            tc.tile_pool(name="dramp", bufs=1, space="DRAM") as dramp,
            tc.tile_pool(name="wpa", bufs=wbufs) as wpa,
            tc.tile_pool(name="wpb", bufs=wbufs) as wpb,
        ):
            xt_sb = constp.tile([128, NKT, BS], f8)
            nc.sync.dma_start(
                xt_sb[:], xt_d[:, :].rearrange("p (kt b) -> p kt b", kt=NKT))
            xtp_sb = constp.tile([128, 2, W_DIM], f32)
            nc.scalar.dma_start(
                xtp_sb[:], xtp_d[:, :].rearrange("(g p) w -> p g w", g=2))
            ones = constp.tile([1, BS], bf16)
            nc.gpsimd.memset(ones[:], 1.0)
            b_sb = []
            for t in range(3):
                bt = constp.tile([1, CPC], bf16, name=f"bias{t}")
                nc.scalar.dma_start(bt[:], b_d[t][:, :])
                b_sb.append(bt)

            qk_sb = sbp.tile([BS, 2 * CPC], bf16, name="qk_sb", tag="qk_sb")
            v_sb = sbp.tile([BS, CPC], bf16, name="v_sb", tag="v_sb")
            a2a1_in = dramp.tile([N_CORES, 2, BPC, CPC], bf16,
                                 tag="a2a1_in", name="a2a1_in")
            a2a1_out = dramp.tile([N_CORES, 2, BPC, CPC], bf16,
                                  tag="a2a1_out", name="a2a1_out")
            a2a2_in = dramp.tile([N_CORES, BPC, CPC], bf16,
                                 tag="a2a2_in", name="a2a2_in")
            a2a2_out = dramp.tile([N_CORES, BPC, CPC], bf16,
                                  tag="a2a2_out", name="a2a2_out")

            def w_chunk_matmuls(m, psum, wt):
                for jj in range(0, CHUNK, 2):
                    for h in range(2):
                        nc.tensor.matmul(
                            psum[:, h * 512:(h + 1) * 512],
                            xt_sb[:, CHUNK * m + jj:CHUNK * m + jj + 2, :],
                            wt[:, jj:jj + 2, h * 512:(h + 1) * 512],
                            start=(m == 0 and jj == 0), stop=False,
                            perf_mode=DR)

            def bias_matmuls(t, psum):
                for h in range(2):
                    nc.tensor.matmul(
                        psum[:, h * 512:(h + 1) * 512],
                        ones[:],
                        b_sb[t][:, h * 512:(h + 1) * 512],
                        start=False, stop=True)

            # ---- phase A: q,k projection ----
            with tc.tile_pool(name="qkps", bufs=1, space="PSUM") as qkps:
                psA = [qkps.tile([BS, CPC], f32, name=f"ps{t}")
                       for t in range(2)]
                for m in range(NCH):
                    for t in range(2):
                        wt = wpa.tile([128, CHUNK, CPC], f8,
                                      tag=f"w{t}", name=f"wt{t}")
                        hwdge[(m * 2 + t) % 2].dma_start(
                            wt[:],
                            w_d[t][128 * m:128 * (m + 1), :].rearrange(
                                "p (j c) -> p j c", j=CHUNK))
                        w_chunk_matmuls(m, psA[t], wt)
                for t in range(2):
                    bias_matmuls(t, psA[t])
                    # permute cols (w d) -> (d w) while leaving PSUM
                    nc.vector.tensor_copy(
                        qk_sb[:, t * CPC:(t + 1) * CPC].rearrange(
                            "b (d w) -> b d w", w=WPC),
                        psA[t].rearrange("b (w d) -> b d w", d=D))

            # stage via SWDGE: the HWDGE rings still hold queued v-weight
            # streaming DMAs, which would head-of-line block the staging
            for j in range(N_CORES):
                nc.gpsimd.dma_start(
                    a2a1_in[j].rearrange("t b c -> b t c"),
                    qk_sb[BPC * j:BPC * (j + 1), :].rearrange(
                        "b (t c) -> b t c", t=2))
            # hold v streaming back until the staging data lands so the
            # A2A1 trigger isn't starved by DMA-engine congestion
            tc.strict_bb_all_engine_barrier()
            nc.gpsimd.collective_compute(
                "AllToAll", mybir.AluOpType.bypass,
                replica_groups=[list(range(N_CORES))],
                ins=[a2a1_in.opt()], outs=[a2a1_out.opt()])

            # ---- phase B: v projection overlapped with attention
            # part A (alphas + exp + denom fold, which need only q,k) ----
            with (
                tc.tile_pool(name="vps", bufs=1, space="PSUM") as vps,
                tc.tile_pool(name="attps", bufs=3, space="PSUM") as attps,
                tc.tile_pool(name="attp", bufs=3) as attp,
                tc.tile_pool(name="keepp", bufs=1) as keepp,
            ):
                eas_tiles = {}

                def emit_part_a(b):
                    qkT = attp.tile([D, 2, N_CORES, WPC], bf16,
                                    tag="qkT", name="qkT")
                    for t in range(2):
                        nc.gpsimd.dma_start(
                            qkT[:, t, :, :],
                            a2a1_out[:, t, b, :].rearrange(
                                "i (d w) -> d i w", d=D))
                    den = attp.tile([128, 4], f32, tag="den", name="den")
                    # HW ACT accum_out accumulates (+=): zero first
                    nc.gpsimd.memset(den[:], 0.0)
                    ea_tiles = []
                    for j in range(2):
                        aT2 = attps.tile([128, 2, 512], f32, tag="aT2",
                                         name="aT2", bufs=2)
                        for h in range(2):
                            kw = 2 * j + h
                            nc.tensor.matmul(
                                aT2[:, h, :], qkT[:, 1, 2 * kw:2 * kw + 2, :],
                                qkT[:, 0, :, :], start=True, stop=True)
                            ea = attp.tile([128, 512], bf16, tag=f"ea{kw}",
                                           name="ea")
                            nc.scalar.activation(
                                ea[:], aT2[:, h, :],
                                mybir.ActivationFunctionType.Exp,
                                scale=EXP_SCALE, accum_out=den[:, kw:kw + 1])
                            ea_tiles.append(ea)
                    rec = attp.tile([128, 4], f32, tag="rec", name="rec")
                    nc.vector.reciprocal(rec[:], den[:])
                    for kw in range(4):
                        # fold softmax denominator AND the 1/64 v-scale
                        eas = keepp.tile([128, 512], bf16,
                                         tag=f"eas{b}_{kw}",
                                         name=f"eas{b}_{kw}")
                        nc.vector.tensor_scalar(
                            eas[:], ea_tiles[kw][:], rec[:, kw:kw + 1],
                            1.0 / WSCALE,
                            op0=mybir.AluOpType.mult,
                            op1=mybir.AluOpType.mult)
                        eas_tiles[(b, kw)] = eas

                psV = vps.tile([BS, CPC], f32, name="psv")
                per = max(1, NCH // BPC)
                next_b = 0
                for m in range(NCH):
                    wt = wpb.tile([128, CHUNK, CPC], f8, tag="w2",
                                  name="wt2")
                    hwdge[m % 2].dma_start(
                        wt[:],
                        w_d[2][128 * m:128 * (m + 1), :].rearrange(
                            "p (j c) -> p j c", j=CHUNK))
                    w_chunk_matmuls(m, psV, wt)
                    while next_b < BPC and next_b <= m // per:
                        emit_part_a(next_b)
                        next_b += 1
                while next_b < BPC:
                    emit_part_a(next_b)
                    next_b += 1
                bias_matmuls(2, psV)
                nc.vector.tensor_copy(v_sb[:], psV[:])

                for j in range(N_CORES):
                    hwdge[j % 2].dma_start(
                        a2a2_in[j], v_sb[BPC * j:BPC * (j + 1), :])
                nc.gpsimd.collective_compute(
                    "AllToAll", mybir.AluOpType.bypass,
                    replica_groups=[list(range(N_CORES))],
                    ins=[a2a2_in.opt()], outs=[a2a2_out.opt()])
                a2a_v = a2a2_out.rearrange("(kw h) b c -> kw h b c", h=2)

                # ---- attention part B: second einsum on pre-scaled
                # alphas; 4 batches share one PSUM bank via 32-aligned
                # quadrants, then one sigmoid + residual add per bank ----
                rT_tiles = []
                for g in range(2):
                    rT = attps.tile([128, W_DIM], f32, tag=f"rT{g}",
                                    name=f"rT{g}", bufs=1)
                    rT_tiles.append(rT)
                for b in range(BPC):
                    vt = attp.tile([128, 4, D], bf16, tag="vt", name="vt")
                    for half in range(2):
                        hwdge[half].dma_start(
                            vt[64 * half:64 * half + 64, :, :],
                            a2a_v[:, half, b, :].rearrange(
                                "i (w d) -> w i d", d=D))
                    rT = rT_tiles[b // 4]
                    q = 32 * (b % 4)
                    for kw in range(4):
                        nc.tensor.matmul(
                            rT[q:q + D, :], vt[:, kw, :],
                            eas_tiles[(b, kw)][:],
                            start=(kw == 0), stop=(kw == 3),
                            tile_position=(0, q))
                for g in range(2):
                    sg = keepp.tile([128, W_DIM], f32, tag=f"sg{g}",
                                    name=f"sg{g}")
                    nc.scalar.activation(
                        sg[:], rT_tiles[g][:],
                        mybir.ActivationFunctionType.Sigmoid)
                    oo = keepp.tile([128, W_DIM], f32, tag=f"oo{g}",
                                    name=f"oo{g}")
                    nc.vector.tensor_add(oo[:], sg[:], xtp_sb[:, g, :])
                    for b4 in range(4):
                        b = 4 * g + b4
                        hwdge[b % 2].dma_start(
                            out_d[D * b:D * (b + 1), :],
                            oo[32 * b4:32 * b4 + D, :])

    nc.compile()
    return nc


def _prep_in_maps(x_in, Wq, bq, Wk, bk, Wv, bv):
    f8 = ml_dtypes.float8_e4m3
    bf16 = ml_dtypes.bfloat16
    x_flat = np.ascontiguousarray(np.asarray(x_in, np.float32).reshape(BS, K))
    # swizzled x^T: [128 p, kt, b] contiguous
    xt = np.ascontiguousarray(
        x_flat.T.reshape(NKT, 128, BS).transpose(1, 0, 2)
    ).reshape(128, NKT * BS).astype(f8)
    # W^T scaled by 64, swizzled to [m, p, j, c] per core slice
    ws = [np.ascontiguousarray(np.asarray(W, np.float32).T) * WSCALE
          for W in (Wq, Wk, Wv)]
    bs = [(np.asarray(b, np.float32) * WSCALE).reshape(1, K).astype(bf16)
          for b in (bq, bk, bv)]
    xtp = np.ascontiguousarray(
        np.asarray(x_in, np.float32).transpose(0, 2, 1))       # (BS, D, W)

    in_maps = []
    for c in range(N_CORES):
        cs = slice(CPC * c, CPC * (c + 1))
        m = {"xt": xt}
        for nm, w in zip(("wq", "wk", "wv"), ws):
            m[nm] = np.ascontiguousarray(
                w[:, cs].reshape(NCH, CHUNK, 128, CPC).transpose(0, 2, 1, 3)
            ).reshape(NCH * 128, CHUNK * CPC).astype(f8)
        for nm, b in zip(("bq", "bk", "bv"), bs):
            m[nm] = np.ascontiguousarray(b[:, cs])
        xp = np.zeros((2, 4, 32, W_DIM), np.float32)
        xp[:, :, :D, :] = xtp[BPC * c:BPC * (c + 1)].reshape(2, 4, D, W_DIM)
        m["xtp"] = xp.reshape(2 * 128, W_DIM)
        in_maps.append(m)
    return in_maps


def _assemble(results):
    out = np.empty((BS, W_DIM, D), np.float32)
    for c in range(N_CORES):
        o = results[c]["out"].reshape(BPC, D, W_DIM)
        out[BPC * c:BPC * (c + 1)] = o.transpose(0, 2, 1)
    return out


def get_nc():
    if "nc" not in _CACHE:
        _CACHE["nc"] = _build()
    return _CACHE["nc"]


def kernel(x_in, Wq, bq, Wk, bk, Wv, bv):
    nc = get_nc()
    in_maps = _prep_in_maps(x_in, Wq, bq, Wk, bk, Wv, bv)
    res = bass_utils.run_bass_kernel_spmd(
        nc, in_maps, core_ids=list(range(N_CORES)))
    return _assemble(res.results)


# revision 28
# speedup vs baseline: 1.1097x; 1.1097x over previous
"""Trainium2 Bass kernel for nn_AttentionBlock (64, 512, 16) / three 8192x8192 Linears.

Strategy (8 NeuronCores, single NEFF, one launch):
  Phase 1 (QKV projection, column-sharded, fp8):
    Each core c owns output columns [1024c, 1024(c+1)) of each Linear
    (= w positions [64c, 64(c+1)), all 16 d). Weights are pre-transposed,
    pre-scaled by 64 (keeps N(0, 0.02^2) values out of e4m3 subnormals)
    and pre-swizzled on host so every 256KB DMA tile is contiguous.
    x is also cast to e4m3. Projection matmuls run in fp8 DoubleRow mode
    (two 128-row k-tiles per instruction). Bias rides a bf16 K=1
    ones-matmul into the same PSUM accumulation. q/k are permuted to
    [b][d][w_local] layout on the way out of PSUM; v stays [b][w_local][d].
  AllToAll: core c sends batch-block j of its q/k (then v) shard to
    core j. After A2A each core holds the full q/k/v for its own 8
    batches -- the attention program is fully static per core.
  Phase 2 (attention, batch-sharded, 8 batches/core):
    alphas are built transposed [k, q] so softmax over the *query* axis
    is a free-dim reduction (exp via ScalarE with fused accum row-sum;
    the 64^2 weight pre-scale folds into the exp input scale). The
    softmax reciprocal AND the leftover 1/64 v-scale fold into the exp'd
    alphas via one two-op tensor_scalar during the overlapped window.
    After the v AllToAll the second einsum consumes v tiles directly as
    matmul lhsT; per-batch results pack into one [128, 512] tile so the
    tail is a single sigmoid + residual add + one 256KB output DMA.
  Host: gathers per-core (128, 512) outputs, transposes back.
"""

import math

import numpy as np
import ml_dtypes

import concourse.bass as bass
import concourse.bacc as bacc
import concourse.mybir as mybir
import concourse.tile as tile
import concourse.bass_utils as bass_utils

N_CORES = 8
BS, W_DIM, D = 64, 512, 16
K = W_DIM * D            # 8192 contraction dim
CPC = K // N_CORES       # 1024 output cols per core
WPC = W_DIM // N_CORES   # 64 w positions per core
BPC = BS // N_CORES      # 8 batches per core
NKT = K // 128           # 64 k-tiles
CHUNK = 8                # k-tiles per weight DMA (4 DoubleRow pairs)
NCH = NKT // CHUNK       # 8 weight chunks
WSCALE = 64.0            # host-side weight pre-scale (fp8 subnormal dodge)
QKSCALE = 8.0            # q/k payload post-scale divisor (fp8 range fit)
EASCALE = 256.0          # exp'd-alpha pre-scale (fp8 subnormal dodge)
EXP_SCALE = QKSCALE * QKSCALE / (math.sqrt(K) * WSCALE * WSCALE)

_CACHE: dict = {}


def _build(wbufs: int = 5):
    f8 = mybir.dt.float8e4
    bf16 = mybir.dt.bfloat16
    f32 = mybir.dt.float32
    DR = mybir.MatmulPerfMode.DoubleRow

    nc = bacc.Bacc("TRN2", target_bir_lowering=False, debug=False,
                   num_devices=N_CORES)

    # xt is host-preswizzled to [128 p, 64 kt, 64 b] so the load is one
    # fully contiguous DMA.
    xt_d = nc.dram_tensor("xt", [128, NKT * BS], f8, kind="ExternalInput")
    # weights host-swizzled to [m, p, j, c] -> each 128-row slice is one
    # contiguous 256KB chunk whose rows are 2KB.
    w_d = [nc.dram_tensor(n, [NCH * 128, CHUNK * CPC], f8,
                          kind="ExternalInput")
           for n in ("wq", "wk", "wv")]
    b_d = [nc.dram_tensor(n, [1, CPC], bf16, kind="ExternalInput")
           for n in ("bq", "bk", "bv")]
    # residual x^T, padded to 32-partition quadrants (4 batches per tile,
    # batch q at partitions [32q, 32q+16))
    xtp_d = nc.dram_tensor("xtp", [2 * 128, W_DIM], f32, kind="ExternalInput")
    out_d = nc.dram_tensor("out", [BPC * D, W_DIM], f32, kind="ExternalOutput")

    hwdge = [nc.sync, nc.scalar]          # the two independent HWDGE rings

    with tile.TileContext(nc) as tc:
        with (
            tc.tile_pool(name="constp", bufs=1) as constp,
            tc.tile_pool(name="sbp", bufs=1) as sbp,
            tc.tile_pool(name="dramp", bufs=1, space="DRAM") as dramp,
            tc.tile_pool(name="wpa", bufs=wbufs) as wpa,
            tc.tile_pool(name="wpb", bufs=wbufs) as wpb,
        ):
            xt_sb = constp.tile([128, NKT, BS], f8)
            nc.sync.dma_start(
                xt_sb[:], xt_d[:, :].rearrange("p (kt b) -> p kt b", kt=NKT))
            xtp_sb = constp.tile([128, 2, W_DIM], f32)
            nc.scalar.dma_start(
                xtp_sb[:], xtp_d[:, :].rearrange("(g p) w -> p g w", g=2))
            ones = constp.tile([1, BS], bf16)
            nc.gpsimd.memset(ones[:], 1.0)
            b_sb = []
            for t in range(3):
                bt = constp.tile([1, CPC], bf16, name=f"bias{t}")
                nc.scalar.dma_start(bt[:], b_d[t][:, :])
                b_sb.append(bt)

            qk_sb = sbp.tile([BS, 2 * CPC], f8, name="qk_sb", tag="qk_sb")
            v_sb = sbp.tile([BS, CPC], f8, name="v_sb", tag="v_sb")
            a2a1_in = dramp.tile([N_CORES, 2, BPC, CPC], f8,
                                 tag="a2a1_in", name="a2a1_in")
            a2a1_out = dramp.tile([N_CORES, 2, BPC, CPC], f8,
                                  tag="a2a1_out", name="a2a1_out")
            a2a2_in = dramp.tile([N_CORES, BPC, CPC], f8,
                                 tag="a2a2_in", name="a2a2_in")
            a2a2_out = dramp.tile([N_CORES, BPC, CPC], f8,
                                  tag="a2a2_out", name="a2a2_out")

            def w_chunk_matmuls(m, psum, wt):
                for jj in range(0, CHUNK, 2):
                    for h in range(2):
                        nc.tensor.matmul(
                            psum[:, h * 512:(h + 1) * 512],
                            xt_sb[:, CHUNK * m + jj:CHUNK * m + jj + 2, :],
                            wt[:, jj:jj + 2, h * 512:(h + 1) * 512],
                            start=(m == 0 and jj == 0), stop=False,
                            perf_mode=DR)

            def bias_matmuls(t, psum):
                for h in range(2):
                    nc.tensor.matmul(
                        psum[:, h * 512:(h + 1) * 512],
                        ones[:],
                        b_sb[t][:, h * 512:(h + 1) * 512],
                        start=False, stop=True)

            # ---- phase A: q,k projection ----
            with tc.tile_pool(name="qkps", bufs=1, space="PSUM") as qkps:
                psA = [qkps.tile([BS, CPC], f32, name=f"ps{t}")
                       for t in range(2)]
                for m in range(NCH):
                    for t in range(2):
                        wt = wpa.tile([128, CHUNK, CPC], f8,
                                      tag=f"w{t}", name=f"wt{t}")
                        hwdge[(m * 2 + t) % 2].dma_start(
                            wt[:],
                            w_d[t][128 * m:128 * (m + 1), :].rearrange(
                                "p (j c) -> p j c", j=CHUNK))
                        w_chunk_matmuls(m, psA[t], wt)
                for t in range(2):
                    bias_matmuls(t, psA[t])
                    # permute cols (w d) -> (d w) while leaving PSUM;
                    # scale into fp8 payload range
                    nc.vector.tensor_scalar_mul(
                        qk_sb[:, t * CPC:(t + 1) * CPC].rearrange(
                            "b (d w) -> b d w", w=WPC),
                        psA[t].rearrange("b (w d) -> b d w", d=D),
                        1.0 / QKSCALE)

            # stage via SWDGE: the HWDGE rings still hold queued v-weight
            # streaming DMAs, which would head-of-line block the staging
            for j in range(N_CORES):
                nc.gpsimd.dma_start(
                    a2a1_in[j].rearrange("t b c -> b t c"),
                    qk_sb[BPC * j:BPC * (j + 1), :].rearrange(
                        "b (t c) -> b t c", t=2))
            # hold v streaming back until the staging data lands so the
            # A2A1 trigger isn't starved by DMA-engine congestion
            tc.strict_bb_all_engine_barrier()
            nc.gpsimd.collective_compute(
                "AllToAll", mybir.AluOpType.bypass,
                replica_groups=[list(range(N_CORES))],
                ins=[a2a1_in.opt()], outs=[a2a1_out.opt()])

            # ---- phase B: v projection overlapped with attention
            # part A (alphas + exp + denom fold, which need only q,k) ----
            with (
                tc.tile_pool(name="vps", bufs=1, space="PSUM") as vps,
                tc.tile_pool(name="attps", bufs=3, space="PSUM") as attps,
                tc.tile_pool(name="attp", bufs=3) as attp,
                tc.tile_pool(name="keepp", bufs=1) as keepp,
            ):
                eas_tiles = {}
                qkT_tiles = []
                den_tiles = []

                def emit_gathers():
                    # all dens + gathers up-front: memsets run under the
                    # collective, gathers fire the moment it completes,
                    # and the exp pipeline never stalls on a fetch
                    for b in range(BPC):
                        den = keepp.tile([128, 4], f32, tag=f"den{b}",
                                         name=f"den{b}")
                        # HW ACT accum_out accumulates (+=): zero first
                        nc.gpsimd.memset(den[:], 0.0)
                        den_tiles.append(den)
                    for b in range(BPC):
                        qkT = keepp.tile([D, 2, N_CORES, WPC], f8,
                                         tag=f"qkT{b}", name=f"qkT{b}")
                        for t in range(2):
                            nc.gpsimd.dma_start(
                                qkT[:, t, :, :],
                                a2a1_out[:, t, b, :].rearrange(
                                    "i (d w) -> d i w", d=D))
                        qkT_tiles.append(qkT)

                def emit_part_a(b):
                    qkT = qkT_tiles[b]
                    den = den_tiles[b]
                    ea_tiles = []
                    for j in range(2):
                        aT2 = attps.tile([128, 2, 512], f32, tag="aT2",
                                         name="aT2", bufs=2)
                        for h in range(2):
                            kw = 2 * j + h
                            nc.tensor.matmul(
                                aT2[:, h, :], qkT[:, 1, 2 * kw:2 * kw + 2, :],
                                qkT[:, 0, :, :], start=True, stop=True)
                            ea = attp.tile([128, 512], bf16, tag=f"ea{kw}",
                                           name="ea")
                            nc.scalar.activation(
                                ea[:], aT2[:, h, :],
                                mybir.ActivationFunctionType.Exp,
                                scale=EXP_SCALE, accum_out=den[:, kw:kw + 1])
                            ea_tiles.append(ea)
                    rec = attp.tile([128, 4], f32, tag="rec", name="rec")
                    nc.vector.reciprocal(rec[:], den[:])
                    for j in range(2):
                        # fold softmax denominator; x256 keeps the near-
                        # uniform softmax weights in fp8 normal range
                        eas = keepp.tile([128, 2, 512], f8,
                                         tag=f"eas{b}_{j}",
                                         name=f"eas{b}_{j}")
                        for h in range(2):
                            kw = 2 * j + h
                            nc.vector.tensor_scalar(
                                eas[:, h, :], ea_tiles[kw][:],
                                rec[:, kw:kw + 1], EASCALE,
                                op0=mybir.AluOpType.mult,
                                op1=mybir.AluOpType.mult)
                        eas_tiles[(b, j)] = eas

                psV = vps.tile([BS, CPC], f32, name="psv")
                emit_gathers()
                per = max(1, NCH // BPC)
                next_b = 0
                for m in range(NCH):
                    wt = wpb.tile([128, CHUNK, CPC], f8, tag="w2",
                                  name="wt2")
                    hwdge[m % 2].dma_start(
                        wt[:],
                        w_d[2][128 * m:128 * (m + 1), :].rearrange(
                            "p (j c) -> p j c", j=CHUNK))
                    w_chunk_matmuls(m, psV, wt)
                    while next_b < BPC and next_b <= m // per:
                        emit_part_a(next_b)
                        next_b += 1
                while next_b < BPC:
                    emit_part_a(next_b)
                    next_b += 1
                bias_matmuls(2, psV)
                nc.vector.tensor_scalar_mul(v_sb[:], psV[:], 1.0 / WSCALE)

                for j in range(N_CORES):
                    hwdge[j % 2].dma_start(
                        a2a2_in[j], v_sb[BPC * j:BPC * (j + 1), :])
                nc.gpsimd.collective_compute(
                    "AllToAll", mybir.AluOpType.bypass,
                    replica_groups=[list(range(N_CORES))],
                    ins=[a2a2_in.opt()], outs=[a2a2_out.opt()])
                a2a_v = a2a2_out.rearrange("(kw h) b c -> kw h b c", h=2)

                # ---- attention part B: second einsum on pre-scaled
                # alphas; 4 batches share one PSUM bank via 32-aligned
                # quadrants, then one sigmoid + residual add per bank ----
                sg_tiles = [keepp.tile([128, W_DIM], f32, tag=f"sg{g}",
                                       name=f"sg{g}") for g in range(2)]
                for b in range(BPC):
                    vt = attp.tile([128, 4, D], f8, tag="vt", name="vt")
                    for half in range(2):
                        hwdge[half].dma_start(
                            vt[64 * half:64 * half + 64, :, :],
                            a2a_v[:, half, b, :].rearrange(
                                "i (w d) -> w i d", d=D))
                    rT = attps.tile([D, W_DIM], f32, tag="rT", name="rT",
                                    bufs=2)
                    for j in range(2):
                        # fp8 DoubleRow: the two kw blocks of a pair ride
                        # as the two reduction k-tiles
                        nc.tensor.matmul(
                            rT[:], vt[:, 2 * j:2 * j + 2, :],
                            eas_tiles[(b, j)][:],
                            start=(j == 0), stop=(j == 1),
                            perf_mode=DR)
                    # per-batch sigmoid straight from PSUM into the
                    # 32-aligned quadrant of the packed output tile
                    nc.scalar.activation(
                        sg_tiles[b // 4][32 * (b % 4):32 * (b % 4) + D, :],
                        rT[:], mybir.ActivationFunctionType.Sigmoid,
                        scale=1.0 / EASCALE)
                for g in range(2):
                    oo = keepp.tile([128, W_DIM], f32, tag=f"oo{g}",
                                    name=f"oo{g}")
                    nc.vector.tensor_add(oo[:], sg_tiles[g][:], xtp_sb[:, g, :])
                    for b4 in range(4):
                        b = 4 * g + b4
                        hwdge[b % 2].dma_start(
                            out_d[D * b:D * (b + 1), :],
                            oo[32 * b4:32 * b4 + D, :])

    nc.compile()
    return nc


def _prep_in_maps(x_in, Wq, bq, Wk, bk, Wv, bv):
    f8 = ml_dtypes.float8_e4m3
    bf16 = ml_dtypes.bfloat16
    x_flat = np.ascontiguousarray(np.asarray(x_in, np.float32).reshape(BS, K))
    # swizzled x^T: [128 p, kt, b] contiguous
    xt = np.ascontiguousarray(
        x_flat.T.reshape(NKT, 128, BS).transpose(1, 0, 2)
    ).reshape(128, NKT * BS).astype(f8)
    # W^T scaled by 64, swizzled to [m, p, j, c] per core slice
    ws = [np.ascontiguousarray(np.asarray(W, np.float32).T) * WSCALE
          for W in (Wq, Wk, Wv)]
    bs = [(np.asarray(b, np.float32) * WSCALE).reshape(1, K).astype(bf16)
          for b in (bq, bk, bv)]
    xtp = np.ascontiguousarray(
        np.asarray(x_in, np.float32).transpose(0, 2, 1))       # (BS, D, W)

    in_maps = []
    for c in range(N_CORES):
        cs = slice(CPC * c, CPC * (c + 1))
        m = {"xt": xt}
        for nm, w in zip(("wq", "wk", "wv"), ws):
            m[nm] = np.ascontiguousarray(
                w[:, cs].reshape(NCH, CHUNK, 128, CPC).transpose(0, 2, 1, 3)
            ).reshape(NCH * 128, CHUNK * CPC).astype(f8)
        for nm, b in zip(("bq", "bk", "bv"), bs):
            m[nm] = np.ascontiguousarray(b[:, cs])
        xp = np.zeros((2, 4, 32, W_DIM), np.float32)
        xp[:, :, :D, :] = xtp[BPC * c:BPC * (c + 1)].reshape(2, 4, D, W_DIM)
        m["xtp"] = xp.reshape(2 * 128, W_DIM)
        in_maps.append(m)
    return in_maps


def _assemble(results):
    out = np.empty((BS, W_DIM, D), np.float32)
    for c in range(N_CORES):
        o = results[c]["out"].reshape(BPC, D, W_DIM)
        out[BPC * c:BPC * (c + 1)] = o.transpose(0, 2, 1)
    return out


def get_nc():
    if "nc" not in _CACHE:
        _CACHE["nc"] = _build()
    return _CACHE["nc"]


def kernel(x_in, Wq, bq, Wk, bk, Wv, bv):
    nc = get_nc()
    in_maps = _prep_in_maps(x_in, Wq, bq, Wk, bk, Wv, bv)
    res = bass_utils.run_bass_kernel_spmd(
        nc, in_maps, core_ids=list(range(N_CORES)))
    return _assemble(res.results)
